# revision 2
# baseline (speedup 1.0000x reference)
"""Block-causal attention (BlockDiffusionDecoder) on 8 TRN2 NeuronCores, v3.

Reference computes, per (b, h):
    S = (Q K^T) / 8, masked so query block i (64 rows) attends key blocks <= i,
    O = softmax(S) V,   shapes [2, 16, 2048, 64] f32.

Sharding: batch*heads (32) split across 8 cores, 4 heads per core, no comm.

v3 design (vs v2):
  The v2 kernel was exp-engine-bound, not PE-bound: every score element
  must leave PSUM through ScalarE (exp) or VectorE (int16 Schraudolph),
  and their combined throughput (153.6 + 122.9 G elem/s) is BELOW the PE
  matmul rate for this shape. Measured per-instruction costs on HW:
  ACT exp [128,1536] = 1571 ns (0.833 ns/col + ~280 ns/instr overhead),
  DVE tensor_scalar [128,1536] PSUM = 1785 ns (1.042 ns/col + ~170 ns).

  v2 split each PSUM score group column-wise between ScalarE and VectorE
  (both engines touch every group; a group's PSUM buffer recycles only
  after BOTH finish). v3 assigns each WHOLE group to ONE engine
  (alternating), so the two ps buffers drain in parallel, instructions
  are fewer and bigger, and each buffer is freed by a single engine.
  Measured ~12 us/rep faster than v2 on HW (interleaved A/B, R=192).

  Everything else follows v2:
  - Host-side layout prep: qkT [NP, 128, 2S] bf16 (partitions 0:64
    head-even, 64:128 head-odd; cols 0:S Q^T, S:2S K^T) and
    vm [NP, 128, 2, 16, 65] bf16 (V plus a ones column).
  - Scores transposed (S^T [k, q]): stationary K^T_j [64, 128], moving
    Q^T [64, 256], heads row-tiled on disjoint partition halves.
    PSUM score groups [128, 1536] (3 k-tiles x 256 q x 2 heads), 2 bufs.
  - DVE groups use int16 Schraudolph (int16 bits == bf16 exp) written
    packed into the P^T tile; ScalarE groups use exact exp (scale=1/8).
  - PV: stationary P^T [128 k, 128 q] bf16 (FWL), moving V'_j [128, 65]
    (ones col -> row sums in col 64); accumulates O[q, d] in PSUM.
  - Output: batched reciprocal + broadcast multiply on VectorE, issued
    one item late; DMA out per (pair, step).
  - Masked 64x64 corners zeroed on the Pool engine.
  - First q/k load split into chunks so QK can start early.
"""

import numpy as np

B, H, S, D = 2, 16, 2048, 64
N_CORES = 8
HPC = (B * H) // N_CORES  # heads per core = 4
NP = HPC // 2  # head pairs per core = 2
QP = S // 256  # q-steps (two 128-row q-tiles per step) = 8
GW = 1536  # PSUM score-group width: 3 k-tiles x 256 q, both heads

# int16 Schraudolph: exp(0.125*s) ~= bitcast_bf16(int16(s*A16 + B16))
SCH_A16 = 128.0 * 1.4426950408889634 * 0.125
SCH_B16 = 127.0 * 128.0 - 334700.0 / 65536.0

# DG[t]: indices of the score groups of q-step t whose exp runs on
# VectorE (whole-group int16 Schraudolph); the rest run exact exp on
# ScalarE. Strict within-step alternation (A,D,A,D,...) keeps both
# engines draining concurrently; ~30 of 72 k-tiles per pair on VectorE.
DG = {0: (), 1: (1,), 2: (1,), 3: (1,), 4: (1, 3), 5: (1, 3),
      6: (1, 3), 7: (1, 3, 5)}

_CACHE = {}


def _build(reps=1):
    import concourse.bass as bass
    import concourse.mybir as mybir
    from concourse import bacc
    from concourse.bass import ts
    from concourse.tile import TileContext
    from concourse.tile_rust import add_dep_helper

    f32 = mybir.dt.float32
    bf16 = mybir.dt.bfloat16
    i16 = mybir.dt.int16

    nc = bacc.Bacc("TRN2", target_bir_lowering=False, debug=False,
                   num_devices=N_CORES)
    qkT = nc.declare_dram_parameter("qkT", [NP, 128, 2 * S], bf16,
                                    isOutput=False)
    vm = nc.declare_dram_parameter("vm", [NP, 128, 2, 16, 65], bf16,
                                   isOutput=False)
    out = nc.declare_dram_parameter("out", [HPC, S, D], f32, isOutput=True)

    def off(j, e):  # free offset of k-tile j (head-half e) inside pT
        return (j // 3) * GW + e * 768 + (j % 3) * 256

    chain_prev = [None]

    def chained(bi):
        if chain_prev[0] is not None:
            add_dep_helper(bi.ins, chain_prev[0].ins, sync=False,
                           reason="dma need-order")
        chain_prev[0] = bi
        return bi

    with TileContext(nc) as tc:
        with (
            tc.tile_pool(name="qkT", bufs=2) as qkT_pool,
            tc.tile_pool(name="vsb", bufs=2) as v_pool,
            tc.tile_pool(name="pT", bufs=3) as pT_pool,
            tc.tile_pool(name="osb", bufs=3) as o_pool,
            tc.tile_pool(name="rec", bufs=3) as r_pool,
            tc.tile_pool(name="ps", bufs=2, space="PSUM") as ps_pool,
            tc.tile_pool(name="ops", bufs=2, space="PSUM") as o_ps_pool,
        ):
            qT = {}
            kT = {}
            v_sb = {}

            def emit_pair_load(p, split):
                qk = qkT_pool.tile([128, 2 * S], bf16, name=f"qkT{p}",
                                   tag="qkT")
                if split:
                    W = 1024
                    for c0, c1 in ((S, S + W), (0, W), (S + W, 2 * S),
                                   (W, S)):
                        chained(nc.sync.dma_start(out=qk[:, c0:c1],
                                                  in_=qkT[p][:, c0:c1]))
                else:
                    chained(nc.sync.dma_start(out=qk[:], in_=qkT[p]))
                qT[p] = qk[:, 0:S]
                kT[p] = qk[:, S:2 * S]
                vs = v_pool.tile([128, 2, 16, 65], bf16, name=f"v{p}", tag="v")
                chained(nc.sync.dma_start(out=vs[:], in_=vm[p]))
                v_sb[p] = vs

            def emit_scores(p, t):
                """Both heads of pair p, q-step t -> P^T in a pT tile."""
                jmax = 2 * t + 1
                ngroups = (jmax + 3) // 3
                pT = pT_pool.tile([128, 6 * GW], bf16, name=f"pT_{p}_{t}",
                                  tag="pT")
                for g in range(ngroups):
                    jn = min(3, jmax + 1 - 3 * g)
                    ps = ps_pool.tile([128, GW], f32,
                                      name=f"ps_{p}_{t}_{g}", tag="ps")
                    for jj in range(jn):
                        j = 3 * g + jj
                        half = 128 if j == jmax else 0
                        for e in range(2):  # heads row-tiled on halves
                            nc.tensor.matmul(
                                ps[:, e * 768 + jj * 256 + half:
                                   e * 768 + (jj + 1) * 256],
                                kT[p][64 * e:64 * e + 64, ts(j, 128)],
                                qT[p][64 * e:64 * e + 64,
                                      t * 256 + half:(t + 1) * 256],
                                start=True, stop=True,
                            )
                    if g in DG[t]:
                        if jn == 3:
                            nc.vector.tensor_scalar(
                                pT[:, g * GW:(g + 1) * GW].bitcast(i16),
                                ps[:], SCH_A16, SCH_B16,
                                mybir.AluOpType.mult, mybir.AluOpType.add)
                        else:
                            dst = pT[:, g * GW:(g + 1) * GW].rearrange(
                                "q (e r) -> q e r", e=2)
                            src = ps[:].rearrange("q (e r) -> q e r", e=2)
                            nc.vector.tensor_scalar(
                                dst[:, :, 0:jn * 256].bitcast(i16),
                                src[:, :, 0:jn * 256], SCH_A16, SCH_B16,
                                mybir.AluOpType.mult, mybir.AluOpType.add)
                    else:
                        if jn == 3:
                            nc.scalar.activation(
                                pT[:, g * GW:(g + 1) * GW], ps[:],
                                mybir.ActivationFunctionType.Exp,
                                scale=0.125)
                        else:
                            dst = pT[:, g * GW:(g + 1) * GW].rearrange(
                                "q (e r) -> q e r", e=2)
                            src = ps[:].rearrange("q (e r) -> q e r", e=2)
                            nc.scalar.activation(
                                dst[:, :, 0:jn * 256],
                                src[:, :, 0:jn * 256],
                                mybir.ActivationFunctionType.Exp,
                                scale=0.125)
                # zero masked 64x64 corners of the two diagonal tiles
                for jc, c0 in ((2 * t, 0), (jmax, 128)):
                    for e in range(2):
                        o = off(jc, e)
                        nc.gpsimd.memset(pT[64:128, o + c0:o + c0 + 64], 0.0)
                return pT

            def emit_pv(p, t, pT):
                o_ps = o_ps_pool.tile([128, 260], f32, name=f"ops_{p}_{t}",
                                      tag="ops")
                for e in range(2):
                    for iq in range(2):  # q-tile 2t+iq
                        i = 2 * t + iq
                        base = 65 * (2 * e + iq)
                        for j in range(i + 1):
                            nc.tensor.matmul(
                                o_ps[:, base:base + 65],
                                pT[:, off(j, e) + 128 * iq:
                                   off(j, e) + 128 * iq + 128],
                                v_sb[p][:, e, j, :],
                                start=(j == 0), stop=(j == i),
                            )
                return o_ps

            def emit_norm(p, t, o_ps):
                rec = r_pool.tile([128, 4], f32, name=f"rec_{p}_{t}",
                                  tag="rec")
                osb = o_pool.tile([128, 2, 2, 64], f32, name=f"osb_{p}_{t}",
                                  tag="osb")
                opsv = o_ps[:].rearrange("q (r c) -> q r c", c=65)
                nc.vector.reciprocal(rec[:], opsv[:, :, 64])
                nc.vector.tensor_tensor(
                    osb[:],
                    opsv[:, :, 0:64].rearrange("q (e i) c -> q e i c", e=2),
                    rec[:].rearrange("q (e i) -> q e i", e=2)
                          .broadcast_to([128, 2, 2, 64]),
                    mybir.AluOpType.mult)
                for e in range(2):
                    nc.sync.dma_start(
                        out=out[2 * p + e, 256 * t:256 * (t + 1)].rearrange(
                            "(n p) d -> p n d", p=128),
                        in_=osb[:, e])

            # ---- software-pipelined issue order ----
            for rep in range(reps):
                pv_pending = None
                norm_q = []
                emit_pair_load(0, split=True)
                # pair 0 ascending (gentle ramp), pair 1 descending so the
                # final item is the smallest step -> short drain tail
                items = [(0, t) for t in range(QP)] + \
                        [(1, t) for t in reversed(range(QP))][:(NP - 1) * QP]
                for p, t in items:
                    pT = emit_scores(p, t)
                    if p == 0 and t == 4 and NP > 1:
                        emit_pair_load(1, split=False)
                    if pv_pending is not None:
                        pp, pt, ppT = pv_pending
                        norm_q.append((pp, pt, emit_pv(pp, pt, ppT)))
                    while norm_q:
                        emit_norm(*norm_q.pop(0))
                    pv_pending = (p, t, pT)
                pp, pt, ppT = pv_pending
                norm_q.append((pp, pt, emit_pv(pp, pt, ppT)))
                for args in norm_q:
                    emit_norm(*args)

    nc.compile()
    return nc


def _get_nc():
    if "nc" not in _CACHE:
        _CACHE["nc"] = _build()
    return _CACHE["nc"]


def prepare_per_core(q, k, v):
    """Host-side layout prep: returns list of 8 per-core input dicts."""
    import ml_dtypes
    bf16 = ml_dtypes.bfloat16

    qf = np.ascontiguousarray(q, dtype=np.float32).reshape(B * H, S, D)
    kf = np.ascontiguousarray(k, dtype=np.float32).reshape(B * H, S, D)
    vf = np.ascontiguousarray(v, dtype=np.float32).reshape(B * H, S, D)

    qT = qf.transpose(0, 2, 1).astype(bf16)  # [BH, 64, S]
    kTt = kf.transpose(0, 2, 1).astype(bf16)

    # qkT[c, p]: [128, 2S]; rows 0:64 head 2p (Q^T | K^T), 64:128 head 2p+1
    qkT = np.empty((B * H // 2, 128, 2 * S), dtype=bf16)
    for pp in range(B * H // 2):
        qkT[pp, 0:64, 0:S] = qT[2 * pp]
        qkT[pp, 0:64, S:] = kTt[2 * pp]
        qkT[pp, 64:128, 0:S] = qT[2 * pp + 1]
        qkT[pp, 64:128, S:] = kTt[2 * pp + 1]
    qkT = qkT.reshape(N_CORES, NP, 128, 2 * S)

    # vm[c, p, kp, e, n, 0:64] = V[head, n*128+kp, :]; [..., 64] = 1
    vm = np.empty((B * H, 16, 128, 65), dtype=bf16)
    vm[:, :, :, 0:64] = vf.reshape(B * H, 16, 128, D).astype(bf16)
    vm[:, :, :, 64] = np.asarray(1.0, dtype=bf16)
    # reorder to [c, p, kp, e, n, 65]
    vm = vm.reshape(N_CORES, NP, 2, 16, 128, 65).transpose(0, 1, 4, 2, 3, 5)
    vm = np.ascontiguousarray(vm)

    return [{"qkT": qkT[c], "vm": vm[c]} for c in range(N_CORES)]


def kernel(q, k, v):
    from concourse.bass_utils import run_bass_kernel_spmd

    nc = _get_nc()
    in_maps = prepare_per_core(q, k, v)
    res = run_bass_kernel_spmd(nc, in_maps, core_ids=list(range(N_CORES)))
    full = np.concatenate([res.results[c]["out"] for c in range(N_CORES)],
                          axis=0)
    return full.reshape(B, H, S, D).astype(np.float32)


# revision 5
# speedup vs baseline: 4.5089x; 4.5089x over previous
"""Block-causal attention (BlockDiffusionDecoder) on 8 TRN2 NeuronCores, v3.

Reference computes, per (b, h):
    S = (Q K^T) / 8, masked so query block i (64 rows) attends key blocks <= i,
    O = softmax(S) V,   shapes [2, 16, 2048, 64] f32.

Sharding: batch*heads (32) split across 8 cores, 4 heads per core, no comm.

v3 design (vs v2):
  The v2 kernel was exp-engine-bound, not PE-bound: every score element
  must leave PSUM through ScalarE (exp) or VectorE (int16 Schraudolph),
  and their combined throughput (153.6 + 122.9 G elem/s) is BELOW the PE
  matmul rate for this shape. Measured per-instruction costs on HW:
  ACT exp [128,1536] = 1571 ns (0.833 ns/col + ~280 ns/instr overhead),
  DVE tensor_scalar [128,1536] PSUM = 1785 ns (1.042 ns/col + ~170 ns).

  v2 split each PSUM score group column-wise between ScalarE and VectorE
  (both engines touch every group; a group's PSUM buffer recycles only
  after BOTH finish). v3 assigns each WHOLE group to ONE engine
  (alternating), so the two ps buffers drain in parallel, instructions
  are fewer and bigger, and each buffer is freed by a single engine.
  Measured ~12 us/rep faster than v2 on HW (interleaved A/B, R=192).

  Everything else follows v2:
  - Host-side layout prep: qkT [NP, 128, 2S] bf16 (partitions 0:64
    head-even, 64:128 head-odd; cols 0:S Q^T, S:2S K^T) and
    vm [NP, 128, 2, 16, 65] bf16 (V plus a ones column).
  - Scores transposed (S^T [k, q]): stationary K^T_j [64, 128], moving
    Q^T [64, 256], heads row-tiled on disjoint partition halves.
    PSUM score groups [128, 1536] (3 k-tiles x 256 q x 2 heads), 2 bufs.
  - DVE groups use int16 Schraudolph (int16 bits == bf16 exp) written
    packed into the P^T tile; ScalarE groups use exact exp (scale=1/8).
  - PV: stationary P^T [128 k, 128 q] bf16 (FWL), moving V'_j [128, 65]
    (ones col -> row sums in col 64); accumulates O[q, d] in PSUM.
  - Output: batched reciprocal + broadcast multiply on VectorE, issued
    one item late; DMA out per (pair, step).
  - Masked 64x64 corners zeroed on the Pool engine.
  - First q/k load split into chunks so QK can start early.
"""

import numpy as np

B, H, S, D = 2, 16, 2048, 64
N_CORES = 8
HPC = (B * H) // N_CORES  # heads per core = 4
NP = HPC // 2  # head pairs per core = 2
QP = S // 256  # q-steps (two 128-row q-tiles per step) = 8
GW = 1536  # PSUM score-group width: 3 k-tiles x 256 q, both heads

# int16 Schraudolph: exp(0.125*s) ~= bitcast_bf16(int16(s*A16 + B16))
SCH_A16 = 128.0 * 1.4426950408889634 * 0.125
SCH_B16 = 127.0 * 128.0 - 334700.0 / 65536.0

# DG[t]: indices of the score groups of q-step t whose exp runs on
# VectorE (whole-group int16 Schraudolph); the rest run exact exp on
# ScalarE. Strict within-step alternation (A,D,A,D,...) keeps both
# engines draining concurrently; ~30 of 72 k-tiles per pair on VectorE.
DG = {0: (), 1: (1,), 2: (1,), 3: (1,), 4: (1, 3), 5: (1, 3),
      6: (1, 3), 7: (1, 3, 5)}

_CACHE = {}


def _build(reps=1):
    import concourse.bass as bass
    import concourse.mybir as mybir
    from concourse import bacc
    from concourse.bass import ts
    from concourse.tile import TileContext
    from concourse.tile_rust import add_dep_helper

    f32 = mybir.dt.float32
    bf16 = mybir.dt.bfloat16
    i16 = mybir.dt.int16

    nc = bacc.Bacc("TRN2", target_bir_lowering=False, debug=False,
                   num_devices=N_CORES)
    qkT = nc.declare_dram_parameter("qkT", [NP, 128, 2 * S], bf16,
                                    isOutput=False)
    vm = nc.declare_dram_parameter("vm", [NP, 128, 2, 16, 65], bf16,
                                   isOutput=False)
    out = nc.declare_dram_parameter("out", [HPC, S, D], f32, isOutput=True)

    def off(j, e):  # free offset of k-tile j (head-half e) inside pT
        return (j // 3) * GW + e * 768 + (j % 3) * 256

    chain_prev = [None]

    def chained(bi):
        if chain_prev[0] is not None:
            add_dep_helper(bi.ins, chain_prev[0].ins, sync=False,
                           reason="dma need-order")
        chain_prev[0] = bi
        return bi

    with TileContext(nc) as tc:
        with (
            tc.tile_pool(name="qkT", bufs=2) as qkT_pool,
            tc.tile_pool(name="vsb", bufs=2) as v_pool,
            tc.tile_pool(name="pT", bufs=3) as pT_pool,
            tc.tile_pool(name="osb", bufs=3) as o_pool,
            tc.tile_pool(name="rec", bufs=3) as r_pool,
            tc.tile_pool(name="ps", bufs=2, space="PSUM") as ps_pool,
            tc.tile_pool(name="ops", bufs=2, space="PSUM") as o_ps_pool,
        ):
            qT = {}
            kT = {}
            v_sb = {}

            def emit_pair_load(p, split):
                qk = qkT_pool.tile([128, 2 * S], bf16, name=f"qkT{p}",
                                   tag="qkT")
                if split:
                    W = 1024
                    for c0, c1 in ((S, S + W), (0, W), (S + W, 2 * S),
                                   (W, S)):
                        chained(nc.sync.dma_start(out=qk[:, c0:c1],
                                                  in_=qkT[p][:, c0:c1]))
                else:
                    chained(nc.sync.dma_start(out=qk[:], in_=qkT[p]))
                qT[p] = qk[:, 0:S]
                kT[p] = qk[:, S:2 * S]
                vs = v_pool.tile([128, 2, 16, 65], bf16, name=f"v{p}", tag="v")
                chained(nc.sync.dma_start(out=vs[:], in_=vm[p]))
                v_sb[p] = vs

            def emit_scores(p, t):
                """Both heads of pair p, q-step t -> P^T in a pT tile."""
                jmax = 2 * t + 1
                ngroups = (jmax + 3) // 3
                pT = pT_pool.tile([128, 6 * GW], bf16, name=f"pT_{p}_{t}",
                                  tag="pT")
                for g in range(ngroups):
                    jn = min(3, jmax + 1 - 3 * g)
                    ps = ps_pool.tile([128, GW], f32,
                                      name=f"ps_{p}_{t}_{g}", tag="ps")
                    for jj in range(jn):
                        j = 3 * g + jj
                        half = 128 if j == jmax else 0
                        for e in range(2):  # heads row-tiled on halves
                            nc.tensor.matmul(
                                ps[:, e * 768 + jj * 256 + half:
                                   e * 768 + (jj + 1) * 256],
                                kT[p][64 * e:64 * e + 64, ts(j, 128)],
                                qT[p][64 * e:64 * e + 64,
                                      t * 256 + half:(t + 1) * 256],
                                start=True, stop=True,
                            )
                    if jn == 1 and 3 * g == jmax:
                        # lone jmax tile: the masked matmul wrote only
                        # q-cols 128:256 of each head slot — exp only the
                        # valid half (the dead half is never read by PV)
                        dst = pT[:, g * GW:(g + 1) * GW].rearrange(
                            "q (e r) -> q e r", e=2)
                        src = ps[:].rearrange("q (e r) -> q e r", e=2)
                        if g in DG[t]:
                            nc.vector.tensor_scalar(
                                dst[:, :, 128:256].bitcast(i16),
                                src[:, :, 128:256], SCH_A16, SCH_B16,
                                mybir.AluOpType.mult, mybir.AluOpType.add)
                        else:
                            nc.scalar.activation(
                                dst[:, :, 128:256], src[:, :, 128:256],
                                mybir.ActivationFunctionType.Exp,
                                scale=0.125)
                    elif g in DG[t]:
                        if jn == 3:
                            nc.vector.tensor_scalar(
                                pT[:, g * GW:(g + 1) * GW].bitcast(i16),
                                ps[:], SCH_A16, SCH_B16,
                                mybir.AluOpType.mult, mybir.AluOpType.add)
                        else:
                            dst = pT[:, g * GW:(g + 1) * GW].rearrange(
                                "q (e r) -> q e r", e=2)
                            src = ps[:].rearrange("q (e r) -> q e r", e=2)
                            nc.vector.tensor_scalar(
                                dst[:, :, 0:jn * 256].bitcast(i16),
                                src[:, :, 0:jn * 256], SCH_A16, SCH_B16,
                                mybir.AluOpType.mult, mybir.AluOpType.add)
                    else:
                        if jn == 3:
                            nc.scalar.activation(
                                pT[:, g * GW:(g + 1) * GW], ps[:],
                                mybir.ActivationFunctionType.Exp,
                                scale=0.125)
                        else:
                            dst = pT[:, g * GW:(g + 1) * GW].rearrange(
                                "q (e r) -> q e r", e=2)
                            src = ps[:].rearrange("q (e r) -> q e r", e=2)
                            nc.scalar.activation(
                                dst[:, :, 0:jn * 256],
                                src[:, :, 0:jn * 256],
                                mybir.ActivationFunctionType.Exp,
                                scale=0.125)
                # zero masked 64x64 corners of the two diagonal tiles
                for jc, c0 in ((2 * t, 0), (jmax, 128)):
                    for e in range(2):
                        o = off(jc, e)
                        nc.gpsimd.memset(pT[64:128, o + c0:o + c0 + 64], 0.0)
                return pT

            def emit_pv(p, t, pT):
                o_ps = o_ps_pool.tile([128, 260], f32, name=f"ops_{p}_{t}",
                                      tag="ops")
                for e in range(2):
                    for iq in range(2):  # q-tile 2t+iq
                        i = 2 * t + iq
                        base = 65 * (2 * e + iq)
                        for j in range(i + 1):
                            nc.tensor.matmul(
                                o_ps[:, base:base + 65],
                                pT[:, off(j, e) + 128 * iq:
                                   off(j, e) + 128 * iq + 128],
                                v_sb[p][:, e, j, :],
                                start=(j == 0), stop=(j == i),
                            )
                return o_ps

            def emit_norm(p, t, o_ps):
                rec = r_pool.tile([128, 4], f32, name=f"rec_{p}_{t}",
                                  tag="rec")
                osb = o_pool.tile([128, 2, 2, 64], f32, name=f"osb_{p}_{t}",
                                  tag="osb")
                opsv = o_ps[:].rearrange("q (r c) -> q r c", c=65)
                nc.vector.reciprocal(rec[:], opsv[:, :, 64])
                nc.vector.tensor_tensor(
                    osb[:],
                    opsv[:, :, 0:64].rearrange("q (e i) c -> q e i c", e=2),
                    rec[:].rearrange("q (e i) -> q e i", e=2)
                          .broadcast_to([128, 2, 2, 64]),
                    mybir.AluOpType.mult)
                for e in range(2):
                    nc.sync.dma_start(
                        out=out[2 * p + e, 256 * t:256 * (t + 1)].rearrange(
                            "(n p) d -> p n d", p=128),
                        in_=osb[:, e])

            # ---- software-pipelined issue order ----
            for rep in range(reps):
                pv_pending = None
                norm_q = []
                emit_pair_load(0, split=True)
                # pair 0 ascending (gentle ramp), pair 1 descending so the
                # final item is the smallest step -> short drain tail
                items = [(0, t) for t in range(QP)] + \
                        [(1, t) for t in reversed(range(QP))][:(NP - 1) * QP]
                for p, t in items:
                    pT = emit_scores(p, t)
                    if p == 0 and t == 4 and NP > 1:
                        emit_pair_load(1, split=False)
                    if pv_pending is not None:
                        pp, pt, ppT = pv_pending
                        norm_q.append((pp, pt, emit_pv(pp, pt, ppT)))
                    while norm_q:
                        emit_norm(*norm_q.pop(0))
                    pv_pending = (p, t, pT)
                pp, pt, ppT = pv_pending
                norm_q.append((pp, pt, emit_pv(pp, pt, ppT)))
                for args in norm_q:
                    emit_norm(*args)

    nc.compile()
    return nc


def _get_nc():
    if "nc" not in _CACHE:
        _CACHE["nc"] = _build()
    return _CACHE["nc"]


def prepare_per_core(q, k, v):
    """Host-side layout prep: returns list of 8 per-core input dicts."""
    import ml_dtypes
    bf16 = ml_dtypes.bfloat16

    qf = np.ascontiguousarray(q, dtype=np.float32).reshape(B * H, S, D)
    kf = np.ascontiguousarray(k, dtype=np.float32).reshape(B * H, S, D)
    vf = np.ascontiguousarray(v, dtype=np.float32).reshape(B * H, S, D)

    qT = qf.transpose(0, 2, 1).astype(bf16)  # [BH, 64, S]
    kTt = kf.transpose(0, 2, 1).astype(bf16)

    # qkT[c, p]: [128, 2S]; rows 0:64 head 2p (Q^T | K^T), 64:128 head 2p+1
    qkT = np.empty((B * H // 2, 128, 2 * S), dtype=bf16)
    for pp in range(B * H // 2):
        qkT[pp, 0:64, 0:S] = qT[2 * pp]
        qkT[pp, 0:64, S:] = kTt[2 * pp]
        qkT[pp, 64:128, 0:S] = qT[2 * pp + 1]
        qkT[pp, 64:128, S:] = kTt[2 * pp + 1]
    qkT = qkT.reshape(N_CORES, NP, 128, 2 * S)

    # vm[c, p, kp, e, n, 0:64] = V[head, n*128+kp, :]; [..., 64] = 1
    vm = np.empty((B * H, 16, 128, 65), dtype=bf16)
    vm[:, :, :, 0:64] = vf.reshape(B * H, 16, 128, D).astype(bf16)
    vm[:, :, :, 64] = np.asarray(1.0, dtype=bf16)
    # reorder to [c, p, kp, e, n, 65]
    vm = vm.reshape(N_CORES, NP, 2, 16, 128, 65).transpose(0, 1, 4, 2, 3, 5)
    vm = np.ascontiguousarray(vm)

    return [{"qkT": qkT[c], "vm": vm[c]} for c in range(N_CORES)]


def kernel(q, k, v):
    from concourse.bass_utils import run_bass_kernel_spmd

    nc = _get_nc()
    in_maps = prepare_per_core(q, k, v)
    res = run_bass_kernel_spmd(nc, in_maps, core_ids=list(range(N_CORES)))
    full = np.concatenate([res.results[c]["out"] for c in range(N_CORES)],
                          axis=0)
    return full.reshape(B, H, S, D).astype(np.float32)
